# revision 1
# baseline (speedup 1.0000x reference)
"""SC-LSTM decoder (2-layer, teacher-forced) Trainium2 Bass kernel.

Strategy (8 NeuronCores):
  - Tensor-parallel over the hidden dimension: core j owns H-rows
    [128j, 128j+128) of each layer (and V-cols [256j, 256j+256) of the
    output projection). Full batch B=128 stays on every core, which
    exactly fills the PE stationary dimension.
  - Phase A (parallel): precompute all x-dependent GEMM contributions
    (gx0 = x@w2h_W0, gx1 = x@w2h_W1[:E], rx = x@w2hr) for all T steps,
    column-sharded across cores.  X is pre-transposed on the host so the
    contraction dim (E) lands on SBUF partitions.
  - Phase C (sequential over T): per step, each core computes its H-slice
    of the gates from SBUF-resident weight slices, the cell update, and
    its new hidden slice; two small AllGathers (64KB/rank) rebuild the
    full transposed hidden state h^T for the next step's contractions.
    The output projection slice runs inside the loop off the gathered
    h^T tiles.
"""

import sys

sys.path.insert(0, "/opt/trn_rl_repo")

import numpy as np

import concourse.bass as bass
import concourse.mybir as mybir
import concourse.tile as tile
from concourse import bacc
from concourse.bass_utils import run_bass_kernel_spmd
from concourse.masks import make_identity

B, T, E, H, D, V, L = 128, 100, 2048, 1024, 256, 2048, 2
NC = 8
P = 128
HS = H // NC      # 128 h-rows per core per layer
GS = 4 * HS       # 512 packed gate cols per core
VS = V // NC      # 256 output cols per core
KE = E // P       # 16 k-tiles over E
KH = H // P       # 8 k-tiles over H
F32 = mybir.dt.float32
F32R = mybir.dt.float32r

_cache = {}


def _build(t_steps: int):
    nc = bacc.Bacc("TRN2", target_bir_lowering=False, debug=False, num_devices=NC)
    BF16 = mybir.dt.bfloat16
    LA = 2  # gx lookahead: gx(t+LA) computed inside step t's AG windows

    # ---------------- I/O declarations (per-core values supplied via in_maps)
    xT = nc.dram_tensor("xT", [E, t_steps * B], F32R, kind="ExternalInput")
    h0T_i = nc.dram_tensor("h0T_i", [H, B], F32R, kind="ExternalInput")
    c_i = nc.dram_tensor("c_i", [B, HS], F32, kind="ExternalInput")
    d_i = nc.dram_tensor("d_i", [B, D], F32, kind="ExternalInput")
    Wx0 = nc.dram_tensor("Wx0", [E, GS], F32R, kind="ExternalInput")
    Wx1x = nc.dram_tensor("Wx1x", [E, GS], F32R, kind="ExternalInput")
    Wrx = nc.dram_tensor("Wrx", [E, 2 * D], F32R, kind="ExternalInput")
    Wh0 = nc.dram_tensor("Wh0", [H, GS], F32R, kind="ExternalInput")
    Wh1 = nc.dram_tensor("Wh1", [H, GS], F32R, kind="ExternalInput")
    Wx1h = nc.dram_tensor("Wx1h", [H, GS], F32R, kind="ExternalInput")
    Wrc = nc.dram_tensor("Wrc", [2 * H, D], F32R, kind="ExternalInput")
    Wr1h = nc.dram_tensor("Wr1h", [H, D], F32R, kind="ExternalInput")
    Wdc0 = nc.dram_tensor("Wdc0", [D, HS], F32R, kind="ExternalInput")
    Wdc1 = nc.dram_tensor("Wdc1", [D, HS], F32R, kind="ExternalInput")
    Wout = nc.dram_tensor("Wout", [2 * H, VS], F32R, kind="ExternalInput")

    out_o = nc.dram_tensor("out", [t_steps, B, VS], F32, kind="ExternalOutput")

    # DRAM scratch for the precomputed r-gate x-contributions
    rxd = nc.dram_tensor("rxd", [t_steps, B, 2 * D], F32)
    gx1d = nc.dram_tensor("gx1d", [t_steps, B, GS], F32)

    rg = [list(range(NC))]

    with tile.TileContext(nc) as tc:
        with tc.tile_pool(name="const", bufs=1) as constp:
            ident = constp.tile([P, P], F32)
            make_identity(nc, ident[:])

            # ---------------- Phase A: precompute rx (r-gate x contributions)
            with (
                tc.tile_pool(name="wxa", bufs=1) as wxap,
                tc.tile_pool(name="xa", bufs=3) as xap,
                tc.tile_pool(name="ga", bufs=3) as gap,
                tc.tile_pool(name="psa", bufs=2, space="PSUM") as psa,
            ):
                wrx = wxap.tile([P, KE, 2 * D], F32R)
                wx1a = wxap.tile([P, KE, GS], F32R)
                nc.sync.dma_start(wrx[:], Wrx.rearrange("(k p) n -> p k n", p=P))
                nc.sync.dma_start(wx1a[:], Wx1x.rearrange("(k p) n -> p k n", p=P))
                for t in range(t_steps):
                    xt = xap.tile([P, KE, B], F32R, tag="xt", name="xt")
                    nc.sync.dma_start(
                        xt[:],
                        xT[:, t * B : (t + 1) * B].rearrange("(k p) n -> p k n", p=P),
                    )
                    rxp = psa.tile([B, 2 * D], F32, tag="rxp", bufs=2, name="rxp")
                    g1xp = psa.tile([B, GS], F32, tag="g1xp", bufs=2, name="g1xp")
                    for k in range(KE):
                        st, sp = (k == 0), (k == KE - 1)
                        nc.tensor.matmul(rxp[:], xt[:, k, :], wrx[:, k, :], start=st, stop=sp)
                        nc.tensor.matmul(g1xp[:], xt[:, k, :], wx1a[:, k, :], start=st, stop=sp)
                    rxs = gap.tile([B, 2 * D], F32, tag="rxs", name="rxs")
                    nc.vector.tensor_copy(rxs[:], rxp[:])
                    nc.sync.dma_start(rxd[t], rxs[:])
                    g1xs = gap.tile([B, GS], F32, tag="g1xs", name="g1xs")
                    nc.vector.tensor_copy(g1xs[:], g1xp[:])
                    nc.sync.dma_start(gx1d[t], g1xs[:])

            # ---------------- Phase B/C: recurrence with interleaved gx GEMMs
            with (
                tc.tile_pool(name="wr", bufs=1) as wrp,
                tc.tile_pool(name="st", bufs=2) as stp,
                tc.tile_pool(name="gx", bufs=3) as gxp,
                tc.tile_pool(name="wk", bufs=2) as wkp,
                tc.tile_pool(name="psg", bufs=2, space="PSUM") as psg,
                tc.tile_pool(name="psr", bufs=2, space="PSUM") as psr,
                tc.tile_pool(name="pst", bufs=2, space="PSUM") as pst,
                tc.tile_pool(name="dma_b", bufs=4, space="DRAM") as dramp,
            ):
                wh0 = wrp.tile([P, KH, GS], F32R)
                wh1 = wrp.tile([P, KH, GS], F32R)
                wx1h = wrp.tile([P, KH, GS], F32R)
                wrc = wrp.tile([P, 2 * KH, D], F32R)
                wr1h = wrp.tile([P, KH, D], F32R)
                wdc0 = wrp.tile([P, D // P, HS], F32R)
                wdc1 = wrp.tile([P, D // P, HS], F32R)
                wout = wrp.tile([P, 2 * KH, VS], F32R)
                wx0 = wrp.tile([P, KE, GS], F32R)
                nc.sync.dma_start(wh0[:], Wh0.rearrange("(k p) n -> p k n", p=P))
                nc.sync.dma_start(wh1[:], Wh1.rearrange("(k p) n -> p k n", p=P))
                nc.sync.dma_start(wx1h[:], Wx1h.rearrange("(k p) n -> p k n", p=P))
                nc.sync.dma_start(wrc[:], Wrc.rearrange("(k p) n -> p k n", p=P))
                nc.sync.dma_start(wr1h[:], Wr1h.rearrange("(k p) n -> p k n", p=P))
                nc.sync.dma_start(wdc0[:], Wdc0.rearrange("(k p) n -> p k n", p=P))
                nc.sync.dma_start(wdc1[:], Wdc1.rearrange("(k p) n -> p k n", p=P))
                nc.sync.dma_start(wout[:], Wout.rearrange("(k p) n -> p k n", p=P))
                nc.sync.dma_start(wx0[:], Wx0.rearrange("(k p) n -> p k n", p=P))

                h0T = stp.tile([P, KH, B], F32R, tag="h0T", name="h0Ti")
                h1T = stp.tile([P, KH, B], F32R, tag="h1T", name="h1Ti")
                nc.sync.dma_start(h0T[:], h0T_i.rearrange("(k p) n -> p k n", p=P))
                nc.sync.dma_start(h1T[:], h0T_i.rearrange("(k p) n -> p k n", p=P))
                c0 = stp.tile([B, HS], F32, tag="c0", name="c0i")
                c1 = stp.tile([B, HS], F32, tag="c1", name="c1i")
                nc.sync.dma_start(c0[:], c_i[:])
                nc.sync.dma_start(c1[:], c_i[:])
                d0 = stp.tile([B, D], F32, tag="d0", name="d0i")
                d1 = stp.tile([B, D], F32, tag="d1", name="d1i")
                nc.sync.dma_start(d0[:], d_i[:])
                nc.sync.dma_start(d1[:], d_i[:])

                Sig = mybir.ActivationFunctionType.Sigmoid
                Tanh = mybir.ActivationFunctionType.Tanh
                mul = mybir.AluOpType.mult
                add = mybir.AluOpType.add

                def load_xt(u):
                    xtb = gxp.tile([P, KE, B], F32R, tag="xtb", bufs=2, name="xtb")
                    nc.sync.dma_start(
                        xtb[:],
                        xT[:, u * B : (u + 1) * B].rearrange("(k p) n -> p k n", p=P),
                    )
                    return xtb

                def gx_compute(xtb, pin_after=None):
                    """In-loop x-contribution GEMMs (bf16) — AG-window filler."""
                    g0x = psr.tile([B, GS], F32, tag="rc0p", bufs=1, name="g0xp")
                    for k in range(KE):
                        m = nc.tensor.matmul(g0x[:], xtb[:, k, :], wx0[:, k, :], start=(k == 0), stop=(k == KE - 1))
                        if k == 0 and pin_after is not None:
                            bass._add_dep_helper(m.ins, pin_after.ins, sync=True, reason="pin filler into AG window")
                    gx0 = gxp.tile([B, GS], F32, tag="gx0", name="gx0")
                    nc.vector.tensor_copy(gx0[:], g0x[:])
                    return gx0

                def gate_act(gp_ap, gx, li, c_cur):
                    gsum = wkp.tile([B, GS], F32, tag=f"gsum{li}", bufs=1, name=f"gsum{li}")
                    nc.vector.tensor_tensor(gsum[:], gp_ap, gx[:], add)
                    sig = wkp.tile([B, 3 * HS], F32, tag=f"sig{li}", bufs=1, name=f"sig{li}")
                    nc.scalar.activation(sig[:], gsum[:, : 3 * HS], Sig)
                    tgc = wkp.tile([B, HS], F32, tag=f"tgc{li}", name=f"tgc{li}")
                    nc.scalar.activation(tgc[:], gsum[:, 3 * HS :], Tanh)
                    cpart = wkp.tile([B, HS], F32, tag=f"cpart{li}", name=f"cpart{li}")
                    nc.vector.tensor_tensor(cpart[:], sig[:, :HS], tgc[:], mul)
                    m2 = wkp.tile([B, HS], F32, tag=f"m2{li}", name=f"m2{li}")
                    nc.vector.tensor_tensor(m2[:], sig[:, HS : 2 * HS], c_cur[:], mul)
                    nc.vector.tensor_tensor(cpart[:], cpart[:], m2[:], add)
                    return gsum, sig, cpart

                def r_dc_path(rsum_in, extra_ps, d_cur, wdc, li):
                    rs = wkp.tile([B, D], F32, tag=f"rsum{li}", bufs=1, name=f"rsum{li}")
                    if extra_ps is not None:
                        nc.vector.tensor_tensor(rs[:], rsum_in, extra_ps, add)
                        nc.scalar.activation(rs[:], rs[:], Sig)
                    else:
                        nc.scalar.activation(rs[:], rsum_in, Sig)
                    d_new = stp.tile([B, D], F32, tag=f"d{li}", name=f"d{li}")
                    nc.vector.tensor_tensor(d_new[:], rs[:], d_cur[:], mul)
                    dtT_p = pst.tile([P, D // P, B], F32, tag="dtTp", bufs=1, name=f"dtTp{li}")
                    for k in range(D // P):
                        nc.tensor.transpose(dtT_p[:, k, :], d_new[:, k * P : (k + 1) * P], ident[:])
                    dtT = wkp.tile([P, D // P, B], F32R, tag=f"dtT{li}", bufs=1, name=f"dtT{li}")
                    nc.vector.tensor_copy(dtT[:], dtT_p[:])
                    dcp = psr.tile([B, HS], F32, tag="dcp", bufs=1, name=f"dcp{li}")
                    for k in range(D // P):
                        nc.tensor.matmul(dcp[:], dtT[:, k, :], wdc[:, k, :], start=(k == 0), stop=(k == D // P - 1))
                    tdc = wkp.tile([B, HS], F32, tag=f"tdc{li}", name=f"tdc{li}")
                    nc.scalar.activation(tdc[:], dcp[:], Tanh)
                    return tdc, d_new

                def finish_cell(cpart, tdc, sig, li):
                    c_new = stp.tile([B, HS], F32, tag=f"c{li}", name=f"c{li}")
                    nc.vector.tensor_tensor(c_new[:], cpart[:], tdc[:], add)
                    nh = wkp.tile([B, HS], F32, tag=f"nh{li}", name=f"nh{li}")
                    nc.scalar.activation(nh[:], c_new[:], Tanh)
                    nc.vector.tensor_tensor(nh[:], sig[:, 2 * HS : 3 * HS], nh[:], mul)
                    return nh, c_new

                def trigger_gather(nh, li):
                    nhT_p = pst.tile([P, B], F32, tag="nhTp", bufs=1, name=f"nhTp{li}")
                    nc.tensor.transpose(nhT_p[:], nh[:], ident[:])
                    nhT = wkp.tile([P, B], F32R, tag=f"nhT{li}", name=f"nhT{li}")
                    nc.vector.tensor_copy(nhT[:], nhT_p[:])
                    agi = dramp.tile([P, B], F32R, tag=f"agi{li}", name=f"agi{li}")
                    ago = dramp.tile([H, B], F32R, tag=f"ago{li}", addr_space="Shared", name=f"ago{li}")
                    nc.sync.dma_start(agi[:], nhT[:])
                    cc = nc.gpsimd.collective_compute(
                        "AllGather", mybir.AluOpType.bypass, replica_groups=rg,
                        ins=[agi[:]], outs=[ago[:]],
                    )
                    return ago, cc

                def load_gathered(ago, li):
                    hT_new = stp.tile([P, KH, B], F32R, tag=f"h{li}T", name=f"h{li}T")
                    nc.sync.dma_start(
                        hT_new[:, : KH // 2, :],
                        ago[: H // 2, :].rearrange("(k p) n -> p k n", p=P),
                    )
                    nc.sync.dma_start(
                        hT_new[:, KH // 2 :, :],
                        ago[H // 2 :, :].rearrange("(k p) n -> p k n", p=P),
                    )
                    return hT_new

                # ---------------- prologue: initial rc parts + gx for steps 0..LA
                rc0p = psr.tile([B, D], F32, tag="rc0p", bufs=1, name="rc0pi")
                for k in range(KH):
                    nc.tensor.matmul(rc0p[:], h0T[:, k, :], wrc[:, k, :], start=(k == 0), stop=(k == KH - 1))
                rc1p = psr.tile([B, D], F32, tag="rc1p", bufs=1, name="rc1pi")
                for k in range(KH):
                    nc.tensor.matmul(rc1p[:], h1T[:, k, :], wrc[:, KH + k, :], start=(k == 0), stop=(k == KH - 1))
                gx_ring = [None] * (LA + 1)
                for u in range(min(LA, t_steps)):
                    gx_ring[u] = gx_compute(load_xt(u))

                out_prev = None
                for t in range(t_steps):
                    rx = gxp.tile([B, 2 * D], F32, tag="rx", bufs=2, name="rx")
                    nc.sync.dma_start(rx[:], rxd[t])
                    gx1 = gxp.tile([B, GS], F32, tag="gx1", bufs=2, name="gx1")
                    nc.sync.dma_start(gx1[:], gx1d[t])
                    gx0 = gx_ring[t % (LA + 1)]

                    # ---- g0(t) + g1 h1-part: tail fillers of the AG#2(t-1) window
                    g0p = psg.tile([B, GS], F32, tag="gout", bufs=1, name="g0p")
                    for k in range(KH):
                        nc.tensor.matmul(g0p[:], h0T[:, k, :], wh0[:, k, :], start=(k == 0), stop=(k == KH - 1))
                    # rc1-part(t): first AG#2(t-1)-dependent PE work
                    rc1p = psr.tile([B, D], F32, tag="rc1p", bufs=1, name="rc1p")
                    for k in range(KH):
                        nc.tensor.matmul(rc1p[:], h1T[:, k, :], wrc[:, KH + k, :], start=(k == 0), stop=(k == KH - 1))
                    g1p = psg.tile([B, GS], F32, tag="g1p", bufs=1, name="g1p")
                    for k in range(KH):
                        nc.tensor.matmul(g1p[:], h1T[:, k, :], wh1[:, k, :], start=(k == 0), stop=False)

                    rpre0 = wkp.tile([B, D], F32, tag="rpre0", bufs=1, name="rpre0")
                    nc.vector.tensor_tensor(rpre0[:], rc0p[:], rx[:, :D], add)
                    rpre1 = wkp.tile([B, D], F32, tag="rpre1", bufs=1, name="rpre1")
                    nc.vector.tensor_tensor(rpre1[:], rc0p[:], rx[:, D:], add)
                    nc.vector.tensor_tensor(rpre0[:], rpre0[:], rc1p[:], add)
                    nc.vector.tensor_tensor(rpre1[:], rpre1[:], rc1p[:], add)

                    gsum0, sig0, cpart0 = gate_act(g0p[:], gx0, 0, c0)
                    tdc0, d0 = r_dc_path(rpre0[:], None, d0, wdc0, 0)
                    nh0, c0 = finish_cell(cpart0, tdc0, sig0, 0)
                    ago0, cc0 = trigger_gather(nh0, 0)

                    # ---- AG#1(t) window fillers: gx(t+LA) + out(t-1)
                    if t + LA < t_steps:
                        gx_ring[(t + LA) % (LA + 1)] = gx_compute(load_xt(t + LA), pin_after=cc0)
                    if out_prev is not None:
                        po0, po1, pt = out_prev
                        outp = psg.tile([B, VS], F32, tag="gout", bufs=1, name="outp")
                        for k in range(2 * KH):
                            src = po0[:, k, :] if k < KH else po1[:, k - KH, :]
                            nc.tensor.matmul(outp[:], src, wout[:, k, :], start=(k == 0), stop=(k == 2 * KH - 1))
                        osb = wkp.tile([B, VS], F32, tag="osb", name="osb")
                        nc.vector.tensor_copy(osb[:], outp[:])
                        nc.sync.dma_start(out_o[pt], osb[:])

                    h0T_new = load_gathered(ago0, 0)

                    # ---- post-AG#1 spine: r1-part + g1 nh0-part
                    r1p = psr.tile([B, D], F32, tag="r1p", bufs=1, name="r1p")
                    for k in range(KH):
                        nc.tensor.matmul(r1p[:], h0T_new[:, k, :], wr1h[:, k, :], start=(k == 0), stop=(k == KH - 1))
                    for k in range(KH):
                        nc.tensor.matmul(g1p[:], h0T_new[:, k, :], wx1h[:, k, :], start=False, stop=(k == KH - 1))

                    gsum1, sig1, cpart1 = gate_act(g1p[:], gx1, 1, c1)
                    tdc1, d1 = r_dc_path(rpre1[:], r1p[:], d1, wdc1, 1)
                    nh1, c1 = finish_cell(cpart1, tdc1, sig1, 1)
                    ago1, cc1 = trigger_gather(nh1, 1)

                    # ---- AG#2(t) window filler: rc0-part(t+1); g0(t+1)/rc1p(t+1)
                    # continue the fill at the top of the next iteration
                    h1T_new = load_gathered(ago1, 1)
                    rc0p = psr.tile([B, D], F32, tag="rc0p", bufs=1, name="rc0p")
                    for k in range(KH):
                        m = nc.tensor.matmul(rc0p[:], h0T_new[:, k, :], wrc[:, k, :], start=(k == 0), stop=(k == KH - 1))
                        if k == 0:
                            bass._add_dep_helper(m.ins, cc1.ins, sync=True, reason="pin filler into AG window")

                    out_prev = (h0T_new, h1T_new, t)
                    h0T, h1T = h0T_new, h1T_new

                po0, po1, pt = out_prev
                outp = psg.tile([B, VS], F32, tag="gout", bufs=1, name="outpF")
                for k in range(2 * KH):
                    src = po0[:, k, :] if k < KH else po1[:, k - KH, :]
                    nc.tensor.matmul(outp[:], src, wout[:, k, :], start=(k == 0), stop=(k == 2 * KH - 1))
                osb = wkp.tile([B, VS], F32, tag="osb", name="osbF")
                nc.vector.tensor_copy(osb[:], outp[:])
                nc.sync.dma_start(out_o[pt], osb[:])

    nc.compile()
    return nc


def _prep_inputs(input_seq, h0, dt0, w2h_W0, w2h_b0, w2h_W1, w2h_b1,
                 w2hr_W0, w2hr_b0, w2hr_W1, w2hr_b1,
                 h2h_W0, h2h_b0, h2h_W1, h2h_b1,
                 h2hr_W0, h2hr_b0, h2hr_W1, h2hr_b1,
                 dc_W0, dc_W1, out_W, out_b, t_steps):
    f = np.float32
    for name, b in [("w2h_b0", w2h_b0), ("w2h_b1", w2h_b1), ("w2hr_b0", w2hr_b0),
                    ("w2hr_b1", w2hr_b1), ("h2h_b0", h2h_b0), ("h2h_b1", h2h_b1),
                    ("h2hr_b0", h2hr_b0), ("h2hr_b1", h2hr_b1), ("out_b", out_b)]:
        assert not np.any(np.asarray(b)), f"nonzero bias {name} unsupported"

    # time-step inputs: SOS one-hot at t=0, then input_seq[:, t-1]
    xs = np.empty((t_steps, B, E), f)
    xs[0] = 0.0
    xs[0, :, 0] = 1.0
    xs[1:] = np.asarray(input_seq, f).transpose(1, 0, 2)[: t_steps - 1]
    xT = np.ascontiguousarray(xs.reshape(t_steps * B, E).T)

    h0 = np.asarray(h0, f)
    h0T = np.ascontiguousarray(h0.T)
    dt0 = np.asarray(dt0, f)

    alpha = 1.0 / L
    wrc_full = np.concatenate([np.asarray(h2hr_W0, f), np.asarray(h2hr_W1, f)], 0) * alpha
    wrx_full = np.concatenate([np.asarray(w2hr_W0, f), np.asarray(w2hr_W1, f)[:E]], 1)

    in_maps = []
    for j in range(NC):
        gc = np.r_[tuple(np.arange(g * H + j * HS, g * H + (j + 1) * HS) for g in range(4))]
        vs = slice(j * VS, (j + 1) * VS)
        in_maps.append({
            "xT": xT,
            "h0T_i": h0T,
            "c_i": np.ascontiguousarray(h0[:, j * HS : (j + 1) * HS]),
            "d_i": dt0,
            "Wx0": np.ascontiguousarray(np.asarray(w2h_W0, f)[:, gc]),
            "Wx1x": np.ascontiguousarray(np.asarray(w2h_W1, f)[:E, gc]),
            "Wrx": wrx_full,
            "Wh0": np.ascontiguousarray(np.asarray(h2h_W0, f)[:, gc]),
            "Wh1": np.ascontiguousarray(np.asarray(h2h_W1, f)[:, gc]),
            "Wx1h": np.ascontiguousarray(np.asarray(w2h_W1, f)[E:, gc]),
            "Wrc": wrc_full,
            "Wr1h": np.ascontiguousarray(np.asarray(w2hr_W1, f)[E:]),
            "Wdc0": np.ascontiguousarray(np.asarray(dc_W0, f)[:, j * HS : (j + 1) * HS]),
            "Wdc1": np.ascontiguousarray(np.asarray(dc_W1, f)[:, j * HS : (j + 1) * HS]),
            "Wout": np.ascontiguousarray(np.asarray(out_W, f)[:, vs]),
        })
    return in_maps


def _run(t_steps, trace, **inputs):
    if trace:
        import prof_shim

        prof_shim.install()
    key = t_steps
    if key not in _cache:
        _cache[key] = _build(t_steps)
    nc = _cache[key]
    in_maps = _prep_inputs(**inputs, t_steps=t_steps)
    res = run_bass_kernel_spmd(nc, in_maps, list(range(NC)), trace=trace)
    parts = [res.results[j]["out"] for j in range(NC)]  # each (T, B, VS)
    full = np.concatenate(parts, axis=2)                # (T, B, V)
    return np.ascontiguousarray(full.transpose(1, 0, 2)), res


def kernel(**inputs) -> np.ndarray:
    out, _ = _run(T, False, **inputs)
    return out


def kernel_traced(t_steps=T, **inputs):
    out, res = _run(t_steps, True, **inputs)
    return out, res



# revision 7
# speedup vs baseline: 1.3229x; 1.3229x over previous
"""SC-LSTM decoder (2-layer, teacher-forced) Trainium2 Bass kernel.

Strategy (8 NeuronCores):
  - Tensor-parallel over the hidden dimension: core j owns H-rows
    [128j, 128j+128) of each layer (and V-cols [256j, 256j+256) of the
    output projection). Full batch B=128 stays on every core, which
    exactly fills the PE stationary dimension.
  - All matmul operands are bf16 (fp32 PSUM accumulation, fp32 cell
    state): on HW fp32r streams at ~2 cycles/row vs bf16's 1.
  - Per step, the x-dependent GEMMs (gx0/gx1/rx for step t+LA) are
    computed inside step t's two AllGather windows straight from a
    single bf16 x-tile load — no separate prologue pass, no DRAM
    round-trip.
  - Two bf16 AllGathers (32KB/rank) per step rebuild the transposed
    hidden state h^T for the next step's contractions; the output
    projection and next-step h-GEMMs fill the rest of the windows.
"""

import sys

sys.path.insert(0, "/opt/trn_rl_repo")

import ml_dtypes
import numpy as np

import concourse.bass as bass
import concourse.mybir as mybir
import concourse.tile as tile
from concourse import bacc
from concourse.bass_utils import run_bass_kernel_spmd
from concourse.masks import make_identity

B, T, E, H, D, V, L = 128, 100, 2048, 1024, 256, 2048, 2
NC = 8
P = 128
HS = H // NC      # 128 h-rows per core per layer
GS = 4 * HS       # 512 packed gate cols per core
VS = V // NC      # 256 output cols per core
KE = E // P       # 16 k-tiles over E
KH = H // P       # 8 k-tiles over H
F32 = mybir.dt.float32
BF16 = mybir.dt.bfloat16
BF = ml_dtypes.bfloat16

_cache = {}


def _build(t_steps: int):
    nc = bacc.Bacc("TRN2", target_bir_lowering=False, debug=False, num_devices=NC)
    LA = 2  # lookahead: x-GEMMs for step t+LA run inside step t's AG windows

    # ---------------- I/O declarations (per-core values supplied via in_maps)
    xT = nc.dram_tensor("xT", [E, t_steps * B], BF16, kind="ExternalInput")
    h0T_i = nc.dram_tensor("h0T_i", [H, B], BF16, kind="ExternalInput")
    c_i = nc.dram_tensor("c_i", [B, HS], F32, kind="ExternalInput")
    d_i = nc.dram_tensor("d_i", [B, D], F32, kind="ExternalInput")
    Wx0 = nc.dram_tensor("Wx0", [E, GS], BF16, kind="ExternalInput")
    Wx1x = nc.dram_tensor("Wx1x", [E, GS], BF16, kind="ExternalInput")
    Wrx = nc.dram_tensor("Wrx", [E, 2 * D], BF16, kind="ExternalInput")
    Wh0 = nc.dram_tensor("Wh0", [H, GS], BF16, kind="ExternalInput")
    Wh1 = nc.dram_tensor("Wh1", [H, GS], BF16, kind="ExternalInput")
    Wx1h = nc.dram_tensor("Wx1h", [H, GS], BF16, kind="ExternalInput")
    Wrc = nc.dram_tensor("Wrc", [2 * H, D], BF16, kind="ExternalInput")
    Wr1h = nc.dram_tensor("Wr1h", [H, D], BF16, kind="ExternalInput")
    Wdc0 = nc.dram_tensor("Wdc0", [D, HS], BF16, kind="ExternalInput")
    Wdc1 = nc.dram_tensor("Wdc1", [D, HS], BF16, kind="ExternalInput")
    Wout = nc.dram_tensor("Wout", [2 * H, VS], BF16, kind="ExternalInput")

    out_o = nc.dram_tensor("out", [t_steps, B, VS], F32, kind="ExternalOutput")

    rg = [list(range(NC))]

    with tile.TileContext(nc) as tc:
        with (
            tc.tile_pool(name="const", bufs=1) as constp,
            tc.tile_pool(name="wr", bufs=1) as wrp,
            tc.tile_pool(name="st", bufs=2) as stp,
            tc.tile_pool(name="gx", bufs=3) as gxp,
            tc.tile_pool(name="wk", bufs=2) as wkp,
            tc.tile_pool(name="psg", bufs=2, space="PSUM") as psg,
            tc.tile_pool(name="psr", bufs=2, space="PSUM") as psr,
            tc.tile_pool(name="psx", bufs=1, space="PSUM") as psx,
            tc.tile_pool(name="pst", bufs=2, space="PSUM") as pst,
            tc.tile_pool(name="dma_b", bufs=4, space="DRAM") as dramp,
        ):
            ident = constp.tile([P, P], F32)
            make_identity(nc, ident[:])

            wh0 = wrp.tile([P, KH, GS], BF16)
            wh1 = wrp.tile([P, KH, GS], BF16)
            wx1h = wrp.tile([P, KH, GS], BF16)
            wrc = wrp.tile([P, 2 * KH, D], BF16)
            wr1h = wrp.tile([P, KH, D], BF16)
            wdc0 = wrp.tile([P, D // P, HS], BF16)
            wdc1 = wrp.tile([P, D // P, HS], BF16)
            wout = wrp.tile([P, 2 * KH, VS], BF16)
            wx0 = wrp.tile([P, KE, GS], BF16)
            wx1a = wrp.tile([P, KE, GS], BF16)
            wrx = wrp.tile([P, KE, 2 * D], BF16)
            nc.sync.dma_start(wh0[:], Wh0.rearrange("(k p) n -> p k n", p=P))
            nc.sync.dma_start(wh1[:], Wh1.rearrange("(k p) n -> p k n", p=P))
            nc.sync.dma_start(wx1h[:], Wx1h.rearrange("(k p) n -> p k n", p=P))
            nc.sync.dma_start(wrc[:], Wrc.rearrange("(k p) n -> p k n", p=P))
            nc.sync.dma_start(wr1h[:], Wr1h.rearrange("(k p) n -> p k n", p=P))
            nc.sync.dma_start(wdc0[:], Wdc0.rearrange("(k p) n -> p k n", p=P))
            nc.sync.dma_start(wdc1[:], Wdc1.rearrange("(k p) n -> p k n", p=P))
            nc.sync.dma_start(wout[:], Wout.rearrange("(k p) n -> p k n", p=P))
            nc.sync.dma_start(wx0[:], Wx0.rearrange("(k p) n -> p k n", p=P))
            nc.sync.dma_start(wx1a[:], Wx1x.rearrange("(k p) n -> p k n", p=P))
            nc.sync.dma_start(wrx[:], Wrx.rearrange("(k p) n -> p k n", p=P))

            h0T = stp.tile([P, KH, B], BF16, tag="h0T", name="h0Ti")
            h1T = stp.tile([P, KH, B], BF16, tag="h1T", name="h1Ti")
            nc.sync.dma_start(h0T[:], h0T_i.rearrange("(k p) n -> p k n", p=P))
            nc.sync.dma_start(h1T[:], h0T_i.rearrange("(k p) n -> p k n", p=P))
            c0 = stp.tile([B, HS], F32, tag="c0", name="c0i")
            c1 = stp.tile([B, HS], F32, tag="c1", name="c1i")
            nc.sync.dma_start(c0[:], c_i[:])
            nc.sync.dma_start(c1[:], c_i[:])
            d0 = stp.tile([B, D], F32, tag="d0", name="d0i")
            d1 = stp.tile([B, D], F32, tag="d1", name="d1i")
            nc.sync.dma_start(d0[:], d_i[:])
            nc.sync.dma_start(d1[:], d_i[:])

            Sig = mybir.ActivationFunctionType.Sigmoid
            Tanh = mybir.ActivationFunctionType.Tanh
            mul = mybir.AluOpType.mult
            add = mybir.AluOpType.add

            def load_xt(u):
                xtb = gxp.tile([P, KE, B], BF16, tag="xtb", bufs=3, name="xtb")
                nc.sync.dma_start(
                    xtb[:],
                    xT[:, u * B : (u + 1) * B].rearrange("(k p) n -> p k n", p=P),
                )
                return xtb

            def x_begin(xtb, pin_after=None):
                """First half of the x-GEMMs (k 0..7), AG#1-window filler."""
                g0x = psx.tile([B, GS], F32, tag="g0xp", bufs=1, name="g0xp")
                g1x = psx.tile([B, GS], F32, tag="g1xp", bufs=1, name="g1xp")
                rxp = psx.tile([B, 2 * D], F32, tag="rxp", bufs=1, name="rxp")
                for k in range(KE // 2):
                    st = k == 0
                    m = nc.tensor.matmul(g0x[:], xtb[:, k, :], wx0[:, k, :], start=st, stop=False)
                    if k == 0 and pin_after is not None:
                        bass._add_dep_helper(m.ins, pin_after.ins, sync=True, reason="pin filler into AG window")
                    nc.tensor.matmul(g1x[:], xtb[:, k, :], wx1a[:, k, :], start=st, stop=False)
                    nc.tensor.matmul(rxp[:], xtb[:, k, :], wrx[:, k, :], start=st, stop=False)
                return g0x, g1x, rxp

            def x_finish(xtb, accs, pin_after=None):
                """Second half of the x-GEMMs (k 8..15), AG#2-window filler."""
                g0x, g1x, rxp = accs
                for k in range(KE // 2, KE):
                    sp = k == KE - 1
                    m = nc.tensor.matmul(g0x[:], xtb[:, k, :], wx0[:, k, :], start=False, stop=sp)
                    if k == KE // 2 and pin_after is not None:
                        bass._add_dep_helper(m.ins, pin_after.ins, sync=True, reason="pin filler into AG window")
                    nc.tensor.matmul(g1x[:], xtb[:, k, :], wx1a[:, k, :], start=False, stop=sp)
                    nc.tensor.matmul(rxp[:], xtb[:, k, :], wrx[:, k, :], start=False, stop=sp)
                gx0 = gxp.tile([B, GS], F32, tag="gx0", bufs=3, name="gx0")
                nc.vector.tensor_copy(gx0[:], g0x[:])
                gx1 = gxp.tile([B, GS], F32, tag="gx1", bufs=3, name="gx1")
                nc.vector.tensor_copy(gx1[:], g1x[:])
                rx = gxp.tile([B, 2 * D], F32, tag="rx", bufs=3, name="rx")
                nc.vector.tensor_copy(rx[:], rxp[:])
                return gx0, gx1, rx

            def gate_act(gp_ap, gx, li, c_cur):
                gsum = wkp.tile([B, GS], F32, tag=f"gsum{li}", bufs=1, name=f"gsum{li}")
                nc.vector.tensor_tensor(gsum[:], gp_ap, gx[:], add)
                sig = wkp.tile([B, 3 * HS], F32, tag=f"sig{li}", bufs=1, name=f"sig{li}")
                nc.scalar.activation(sig[:], gsum[:, : 3 * HS], Sig)
                tgc = wkp.tile([B, HS], F32, tag=f"tgc{li}", name=f"tgc{li}")
                nc.scalar.activation(tgc[:], gsum[:, 3 * HS :], Tanh)
                cpart = wkp.tile([B, HS], F32, tag=f"cpart{li}", name=f"cpart{li}")
                nc.vector.tensor_tensor(cpart[:], sig[:, :HS], tgc[:], mul)
                m2 = wkp.tile([B, HS], F32, tag=f"m2{li}", name=f"m2{li}")
                nc.vector.tensor_tensor(m2[:], sig[:, HS : 2 * HS], c_cur[:], mul)
                nc.vector.tensor_tensor(cpart[:], cpart[:], m2[:], add)
                return gsum, sig, cpart

            def r_dc_path(rsum_in, extra_ps, d_cur, wdc, li):
                rs = wkp.tile([B, D], F32, tag=f"rsum{li}", bufs=1, name=f"rsum{li}")
                if extra_ps is not None:
                    nc.vector.tensor_tensor(rs[:], rsum_in, extra_ps, add)
                    nc.scalar.activation(rs[:], rs[:], Sig)
                else:
                    nc.scalar.activation(rs[:], rsum_in, Sig)
                d_new = stp.tile([B, D], F32, tag=f"d{li}", name=f"d{li}")
                nc.vector.tensor_tensor(d_new[:], rs[:], d_cur[:], mul)
                dtT_p = pst.tile([P, D // P, B], F32, tag="tps", bufs=1, name=f"dtTp{li}")
                for k in range(D // P):
                    nc.tensor.transpose(dtT_p[:, k, :], d_new[:, k * P : (k + 1) * P], ident[:])
                dtT = wkp.tile([P, D // P, B], BF16, tag=f"dtT{li}", bufs=1, name=f"dtT{li}")
                nc.vector.tensor_copy(dtT[:], dtT_p[:])
                dcp = psr.tile([B, HS], F32, tag="rdc", bufs=1, name=f"dcp{li}")
                for k in range(D // P):
                    nc.tensor.matmul(dcp[:], dtT[:, k, :], wdc[:, k, :], start=(k == 0), stop=(k == D // P - 1))
                tdc = wkp.tile([B, HS], F32, tag=f"tdc{li}", name=f"tdc{li}")
                nc.scalar.activation(tdc[:], dcp[:], Tanh)
                return tdc, d_new

            def finish_cell(cpart, tdc, sig, li):
                c_new = stp.tile([B, HS], F32, tag=f"c{li}", name=f"c{li}")
                nc.vector.tensor_tensor(c_new[:], cpart[:], tdc[:], add)
                nh = wkp.tile([B, HS], F32, tag=f"nh{li}", name=f"nh{li}")
                nc.scalar.activation(nh[:], c_new[:], Tanh)
                nc.vector.tensor_tensor(nh[:], sig[:, 2 * HS : 3 * HS], nh[:], mul)
                return nh, c_new

            def trigger_gather(nh, li):
                nhT_p = pst.tile([P, B], F32, tag="tps", bufs=1, name=f"nhTp{li}")
                nc.tensor.transpose(nhT_p[:], nh[:], ident[:])
                nhT = wkp.tile([P, B], BF16, tag=f"nhT{li}", name=f"nhT{li}")
                nc.vector.tensor_copy(nhT[:], nhT_p[:])
                agi = dramp.tile([P, B], BF16, tag=f"agi{li}", name=f"agi{li}")
                ago = dramp.tile([H, B], BF16, tag=f"ago{li}", addr_space="Shared", name=f"ago{li}")
                nc.sync.dma_start(agi[:], nhT[:])
                cc = nc.gpsimd.collective_compute(
                    "AllGather", mybir.AluOpType.bypass, replica_groups=rg,
                    ins=[agi[:]], outs=[ago[:]],
                )
                return ago, cc

            def load_gathered(ago, li):
                hT_new = stp.tile([P, KH, B], BF16, tag=f"h{li}T", name=f"h{li}T")
                nc.sync.dma_start(
                    hT_new[:, : KH // 2, :],
                    ago[: H // 2, :].rearrange("(k p) n -> p k n", p=P),
                )
                nc.sync.dma_start(
                    hT_new[:, KH // 2 :, :],
                    ago[H // 2 :, :].rearrange("(k p) n -> p k n", p=P),
                )
                return hT_new

            # ---------------- prologue: rcs part0 + g0p(0) + x-GEMMs for steps 0..LA-1
            rcs = psr.tile([B, D], F32, tag="rcs", bufs=1, name="rcsi")
            for k in range(KH):
                nc.tensor.matmul(rcs[:], h0T[:, k, :], wrc[:, k, :], start=(k == 0), stop=False)
            g0p = psg.tile([B, GS], F32, tag="gout", bufs=1, name="g0pi")
            for k in range(KH):
                nc.tensor.matmul(g0p[:], h0T[:, k, :], wh0[:, k, :], start=(k == 0), stop=(k == KH - 1))
            gx_ring = [None] * (LA + 1)
            for u in range(min(LA, t_steps)):
                xtb = load_xt(u)
                gx_ring[u] = x_finish(xtb, x_begin(xtb))

            out_prev = None
            for t in range(t_steps):
                gx0, gx1, rx = gx_ring[t % (LA + 1)]

                # ---- post-AG#2(t-1) spine: close rcs with the h1 part + g1 h1-part
                for k in range(KH):
                    nc.tensor.matmul(rcs[:], h1T[:, k, :], wrc[:, KH + k, :], start=False, stop=(k == KH - 1))
                g1p = psg.tile([B, GS], F32, tag="g1p", bufs=1, name="g1p")
                for k in range(KH):
                    nc.tensor.matmul(g1p[:], h1T[:, k, :], wh1[:, k, :], start=(k == 0), stop=False)

                rpre0 = wkp.tile([B, D], F32, tag="rpre0", bufs=1, name="rpre0")
                nc.vector.tensor_tensor(rpre0[:], rcs[:], rx[:, :D], add)
                rpre1 = wkp.tile([B, D], F32, tag="rpre1", bufs=1, name="rpre1")
                nc.vector.tensor_tensor(rpre1[:], rcs[:], rx[:, D:], add)

                gsum0, sig0, cpart0 = gate_act(g0p[:], gx0, 0, c0)
                tdc0, d0 = r_dc_path(rpre0[:], None, d0, wdc0, 0)
                nh0, c0 = finish_cell(cpart0, tdc0, sig0, 0)
                ago0, cc0 = trigger_gather(nh0, 0)

                # ---- AG#1(t) window fillers: first half of x-GEMMs for t+LA
                xtb_next = accs_next = None
                if t + LA < t_steps:
                    xtb_next = load_xt(t + LA)
                    accs_next = x_begin(xtb_next, pin_after=cc0)

                h0T_new = load_gathered(ago0, 0)

                # ---- post-AG#1 spine: r1-part + g1 nh0-part
                r1p = psr.tile([B, D], F32, tag="rdc", bufs=1, name="r1p")
                for k in range(KH):
                    nc.tensor.matmul(r1p[:], h0T_new[:, k, :], wr1h[:, k, :], start=(k == 0), stop=(k == KH - 1))
                for k in range(KH):
                    nc.tensor.matmul(g1p[:], h0T_new[:, k, :], wx1h[:, k, :], start=False, stop=(k == KH - 1))

                gsum1, sig1, cpart1 = gate_act(g1p[:], gx1, 1, c1)
                tdc1, d1 = r_dc_path(rpre1[:], r1p[:], d1, wdc1, 1)
                nh1, c1 = finish_cell(cpart1, tdc1, sig1, 1)
                ago1, cc1 = trigger_gather(nh1, 1)

                # ---- AG#2(t) window fillers: rest of x-GEMMs, rcs(t+1) h0-part,
                # out(t-1), g0p(t+1) — all independent of AG#2(t)
                if xtb_next is not None:
                    gx_ring[(t + LA) % (LA + 1)] = x_finish(xtb_next, accs_next, pin_after=cc1)
                if t + 1 < t_steps:
                    rcs = psr.tile([B, D], F32, tag="rcs", bufs=1, name="rcs")
                    for k in range(KH):
                        nc.tensor.matmul(rcs[:], h0T_new[:, k, :], wrc[:, k, :], start=(k == 0), stop=False)
                if out_prev is not None:
                    po0, po1, pt = out_prev
                    outp = psg.tile([B, VS], F32, tag="gout", bufs=1, name="outp")
                    for k in range(2 * KH):
                        src = po0[:, k, :] if k < KH else po1[:, k - KH, :]
                        nc.tensor.matmul(outp[:], src, wout[:, k, :], start=(k == 0), stop=(k == 2 * KH - 1))
                    osb = wkp.tile([B, VS], F32, tag="osb", name="osb")
                    nc.vector.tensor_copy(osb[:], outp[:])
                    nc.sync.dma_start(out_o[pt], osb[:])
                if t + 1 < t_steps:
                    g0p = psg.tile([B, GS], F32, tag="gout", bufs=1, name="g0p")
                    for k in range(KH):
                        nc.tensor.matmul(g0p[:], h0T_new[:, k, :], wh0[:, k, :], start=(k == 0), stop=(k == KH - 1))

                h1T_new = load_gathered(ago1, 1)

                out_prev = (h0T_new, h1T_new, t)
                h0T, h1T = h0T_new, h1T_new

            po0, po1, pt = out_prev
            outp = psg.tile([B, VS], F32, tag="gout", bufs=1, name="outpF")
            for k in range(2 * KH):
                src = po0[:, k, :] if k < KH else po1[:, k - KH, :]
                nc.tensor.matmul(outp[:], src, wout[:, k, :], start=(k == 0), stop=(k == 2 * KH - 1))
            osb = wkp.tile([B, VS], F32, tag="osb", name="osbF")
            nc.vector.tensor_copy(osb[:], outp[:])
            nc.sync.dma_start(out_o[pt], osb[:])

    nc.compile()
    return nc


def _prep_inputs(input_seq, h0, dt0, w2h_W0, w2h_b0, w2h_W1, w2h_b1,
                 w2hr_W0, w2hr_b0, w2hr_W1, w2hr_b1,
                 h2h_W0, h2h_b0, h2h_W1, h2h_b1,
                 h2hr_W0, h2hr_b0, h2hr_W1, h2hr_b1,
                 dc_W0, dc_W1, out_W, out_b, t_steps):
    f = np.float32
    for name, b in [("w2h_b0", w2h_b0), ("w2h_b1", w2h_b1), ("w2hr_b0", w2hr_b0),
                    ("w2hr_b1", w2hr_b1), ("h2h_b0", h2h_b0), ("h2h_b1", h2h_b1),
                    ("h2hr_b0", h2hr_b0), ("h2hr_b1", h2hr_b1), ("out_b", out_b)]:
        assert not np.any(np.asarray(b)), f"nonzero bias {name} unsupported"

    # time-step inputs: SOS one-hot at t=0, then input_seq[:, t-1]
    xs = np.empty((t_steps, B, E), f)
    xs[0] = 0.0
    xs[0, :, 0] = 1.0
    xs[1:] = np.asarray(input_seq, f).transpose(1, 0, 2)[: t_steps - 1]
    xT = np.ascontiguousarray(xs.reshape(t_steps * B, E).T.astype(BF))

    h0 = np.asarray(h0, f)
    h0T = np.ascontiguousarray(h0.T.astype(BF))
    dt0 = np.asarray(dt0, f)

    alpha = 1.0 / L
    wrc_full = (np.concatenate([np.asarray(h2hr_W0, f), np.asarray(h2hr_W1, f)], 0) * alpha).astype(BF)
    wrx_full = np.concatenate([np.asarray(w2hr_W0, f), np.asarray(w2hr_W1, f)[:E]], 1).astype(BF)

    in_maps = []
    for j in range(NC):
        gc = np.r_[tuple(np.arange(g * H + j * HS, g * H + (j + 1) * HS) for g in range(4))]
        vs = slice(j * VS, (j + 1) * VS)
        in_maps.append({
            "xT": xT,
            "h0T_i": h0T,
            "c_i": np.ascontiguousarray(h0[:, j * HS : (j + 1) * HS]),
            "d_i": dt0,
            "Wx0": np.ascontiguousarray(np.asarray(w2h_W0, f)[:, gc].astype(BF)),
            "Wx1x": np.ascontiguousarray(np.asarray(w2h_W1, f)[:E, gc].astype(BF)),
            "Wrx": wrx_full,
            "Wh0": np.ascontiguousarray(np.asarray(h2h_W0, f)[:, gc].astype(BF)),
            "Wh1": np.ascontiguousarray(np.asarray(h2h_W1, f)[:, gc].astype(BF)),
            "Wx1h": np.ascontiguousarray(np.asarray(w2h_W1, f)[E:, gc].astype(BF)),
            "Wrc": wrc_full,
            "Wr1h": np.ascontiguousarray(np.asarray(w2hr_W1, f)[E:].astype(BF)),
            "Wdc0": np.ascontiguousarray(np.asarray(dc_W0, f)[:, j * HS : (j + 1) * HS].astype(BF)),
            "Wdc1": np.ascontiguousarray(np.asarray(dc_W1, f)[:, j * HS : (j + 1) * HS].astype(BF)),
            "Wout": np.ascontiguousarray(np.asarray(out_W, f)[:, vs].astype(BF)),
        })
    return in_maps


def _run(t_steps, trace, **inputs):
    if trace:
        import prof_shim

        prof_shim.install()
    key = t_steps
    if key not in _cache:
        _cache[key] = _build(t_steps)
    nc = _cache[key]
    in_maps = _prep_inputs(**inputs, t_steps=t_steps)
    res = run_bass_kernel_spmd(nc, in_maps, list(range(NC)), trace=trace)
    parts = [res.results[j]["out"] for j in range(NC)]  # each (T, B, VS)
    full = np.concatenate(parts, axis=2)                # (T, B, V)
    return np.ascontiguousarray(full.transpose(1, 0, 2).astype(np.float32)), res


def kernel(**inputs) -> np.ndarray:
    out, _ = _run(T, False, **inputs)
    return out


def kernel_traced(t_steps=T, **inputs):
    out, res = _run(t_steps, True, **inputs)
    return out, res
